# revision 22
# baseline (speedup 1.0000x reference)
"""DMRG two-site effective Hamiltonian application (ApplyMPO) on 8 trn2 cores.

Math (reference):
  res[h,i,j,k] = sum_{a,b,c,d,e,f,g} L[b,h,a] M1[b,d,i,c] M2[d,f,j,e]
                                     R[f,k,g] psi[a,c,e,g]

Device algorithm (per core, output bond h sharded 8 x 128), all fp16 with
fp32 PSUM accumulation:
  Q[(b,c,e),(i,j,f)] = sum_d M1[b,d,i,c] M2[d,f,j,e]            (host, 400 els)
  step1: T1[h; g, slot32]  = sum_a L[b,h,a] psi[a,(c,e),g]      (PE, K=a)
         slot = b*4 + ce (20 used, 12 zero-padded so 4 g's = 128 cols)
  flipA: T1P[(g4,slot32); w, h] = DMA-xbar transpose, 32 windows per g-block
  mix:   T3H[h; (i,j,f), g]     = T1P[w]^T @ Q4P  (Q4P = I4 (x) Q, padded)
  flipB: T3G[g; (i,j,f), h]     = DMA-xbar transpose (batched, 1 per block)
  step4: res[h; (i,j),k]       += T3G^T @ R^T[f][g,k]  (PE, K=g, 8 PSUM banks
         accumulate across all 8 g-blocks, evacuated once at the end)
Phase A (step1+flip+mix) is a 2-stage software pipeline: mid-section of
quarter q-1 interleaves with step1 of quarter q so PE matmul bursts hide the
DMA-transpose and evacuation latencies. Transposes run on the otherwise-idle
DMA engines (xbar), not the PE.
"""

import numpy as np

import concourse.bacc as bacc
import concourse.mybir as mybir
import concourse.tile as tile
from concourse import bass_utils

F32 = mybir.dt.float32
F16 = mybir.dt.float16

CHI = 1024
W = 5
D = 2
NCORES = 8
H = CHI // NCORES  # 128, h rows per core

_nc_cache = None


def _build_nc():
    nc = bacc.Bacc("TRN2", target_bir_lowering=False)
    # host-prearranged: psi[ac, q, a_lo, ce, g256]; lt[b, a_lo, ac, h]; rt[blk, g_lo, f, k]
    psi = nc.dram_tensor("psi", [8, 4, 128, 4, 256], F16, kind="ExternalInput")
    lt = nc.dram_tensor("lt", [5, 128, 8, H], F16, kind="ExternalInput")
    rt = nc.dram_tensor("rt", [8, 128, 5, 1024], F16, kind="ExternalInput")
    q4 = nc.dram_tensor("q4", [128, 128], F16, kind="ExternalInput")
    res = nc.dram_tensor("res", [H, 4096], F32, kind="ExternalOutput")  # h;(i,j,k)

    with tile.TileContext(nc) as tc:
        with (
            tc.tile_pool(name="const", bufs=1) as const_pool,
            tc.tile_pool(name="psis", bufs=16) as psi_pool,
            tc.tile_pool(name="t1", bufs=2) as t1_pool,
            tc.tile_pool(name="t1p", bufs=3) as t1p_pool,
            tc.tile_pool(name="t3h", bufs=4) as t3h_pool,
            tc.tile_pool(name="t3g", bufs=8) as t3g_pool,
            tc.tile_pool(name="rblk", bufs=4) as rblk_pool,
            tc.tile_pool(name="resst", bufs=3) as res_pool,
        ):
            # ---- static loads (only b=0 weights gate the first matmul) ----
            lt_sb = const_pool.tile([128, 5, 8, H], F16)  # [a_lo; b, ac, h]
            lt_r = lt.ap().rearrange("b p ac h -> p b ac h")
            nc.sync.dma_start(lt_sb[:, 0], lt_r[:, 0])
            q4_sb = const_pool.tile([128, 128], F16)

            def load_rest_of_consts():
                for b in range(1, 5):
                    nc.sync.dma_start(lt_sb[:, b], lt_r[:, b])
                nc.sync.dma_start(q4_sb[:], q4.ap())

            alt_ct = 0

            def evac_copy(out, in_):
                nonlocal alt_ct
                alt_ct += 1
                if alt_ct % 2 == 0:
                    nc.scalar.copy(out, in_)
                else:
                    nc.vector.tensor_copy(out, in_)

            psis = {}     # q -> [8 tiles]
            t1qs = {}     # q -> tile [128, 256, 32]
            t1ps = {}     # (q, blk2) -> tile [128, 32, 128]
            t3hs = {}     # (q, blk2) -> tile [128, 20, 128]
            t3gs = {}     # blk -> tile [128, 20, 128]
            rblks = {}    # blk -> tile [128, 5, 1024]

            def emit_psi_loads(q):
                tiles = []
                for ac in range(8):
                    p = psi_pool.tile([128, 4, 256], F16, tag="psi", name="psi_sl")
                    nc.sync.dma_start(p[:], psi.ap()[ac, q])
                    tiles.append(p)
                psis[q] = tiles

            def emit_rt_load(blk):
                rb = rblk_pool.tile([128, 5, 1024], F16, tag="rblk", name="rblk_t")
                nc.sync.dma_start(rb[:], rt.ap()[blk])
                rblks[blk] = rb

            def xpose(out, in_):
                # NOTE: ACT-dispatched xbar transposes produced garbage on HW
                # (sim-correct); keep every transpose on the SP sequencer.
                nc.sync.dma_start_transpose(out, in_)

            def emit_s1_b(q, b, ps_s1):
                t1q = t1qs[q]
                ps1 = ps_s1.tile([128, 4, 256], F32, tag="s1", name="ps1_t")  # 2 banks
                ps1_flat = ps1[:].rearrange("p c g -> p (c g)")
                for ac in range(8):
                    lhsT = lt_sb[:, b, ac]
                    psi_flat = psis[q][ac][:].rearrange("p c g -> p (c g)")
                    for cep in range(2):  # one 512-wide MM per PSUM bank
                        nc.tensor.matmul(
                            ps1_flat[:, cep * 512:(cep + 1) * 512],
                            lhsT,
                            psi_flat[:, cep * 512:(cep + 1) * 512],
                            start=(ac == 0),
                            stop=(ac == 7),
                        )
                # slot = b*4 + ce; slots 20..31 stay zero (memset at alloc)
                nc.vector.tensor_copy(
                    t1q[:, :, b * 4:b * 4 + 2],
                    ps1[:, 0:2].rearrange("p ce g -> p g ce"),
                )
                nc.scalar.copy(
                    t1q[:, :, b * 4 + 2:b * 4 + 4],
                    ps1[:, 2:4].rearrange("p ce g -> p g ce"),
                )

            def emit_flipA(q):
                # split into 4 xbar pieces per g-block so mixes can start on
                # the first 8 windows instead of waiting for the full block
                t1q = t1qs[q]
                for blk2 in range(2):
                    tp = t1p_pool.tile([128, 32, 128], F16, tag="t1p", name="t1p_t")
                    for pc in range(4):
                        xpose(
                            tp[:, pc * 8:(pc + 1) * 8, :],
                            t1q[:, blk2 * 128 + pc * 32:blk2 * 128 + (pc + 1) * 32, :],
                        )
                    t1ps[(q, blk2)] = tp

            def emit_mix_half(q, blk2, half, ps_pm):
                tp = t1ps[(q, blk2)]
                if half == 0:
                    t3hs[(q, blk2)] = t3h_pool.tile([128, 20, 128], F16, tag="t3h", name="t3h_t")
                t3h = t3hs[(q, blk2)]
                for pair in range(8):  # 2 windows (of 4 g each) per PSUM bank
                    w0 = half * 16 + pair * 2
                    pm = ps_pm.tile([128, 2, 80], F32, tag="pm", name="pm_t")
                    for wi in range(2):
                        nc.tensor.matmul(
                            pm[:, wi],
                            tp[:, w0 + wi, :],
                            q4_sb[:, 0:80],
                            start=True,
                            stop=True,
                        )
                    evac_copy(
                        t3h[:, :, w0 * 4:(w0 + 2) * 4].rearrange(
                            "p i (w g) -> p i w g", w=2
                        ),
                        pm[:].rearrange("p w (i g) -> p i w g", g=4),
                    )

            def emit_flipB(q, blk2):
                blk = q * 2 + blk2
                tg = t3g_pool.tile([128, 20, 128], F16, tag="t3g", name="t3g_t")
                for pc in range(2):
                    xpose(
                        tg[:, pc * 10:(pc + 1) * 10, :],
                        t3hs[(q, blk2)][:, pc * 10:(pc + 1) * 10, :],
                    )
                t3gs[blk] = tg

            # ============ phase A + staged phase-B via side-split PSUM ======
            # s1 lives on PSUM-left, pm on PSUM-right. After stage 3 the s1
            # pool closes and the kh=0 step4 accumulators open on the left, so
            # blocks 0-2 of step4 run while stage 4's mixes trickle behind the
            # flipA(3) xbar. After stage 4 the pm pool closes and the kh=1
            # accumulators take the right side.
            from contextlib import ExitStack as _ES

            def s4_mm(ps4t, blk, ij, f, kh, start, stop):
                nc.tensor.matmul(
                    ps4t[:],
                    t3gs[blk][:, ij * 5 + f, :],
                    rblks[blk][:, f, kh * 512:(kh + 1) * 512],
                    start=start,
                    stop=stop,
                )

            st_s1, st_pm, st_a, st_b = _ES(), _ES(), _ES(), _ES()
            ps_pm = st_pm.enter_context(
                tc.tile_pool(name="ps_pm", bufs=4, space="PSUM", side="right")
            )
            ps_s1 = st_s1.enter_context(
                tc.tile_pool(name="ps_s1", bufs=2, space="PSUM", side="left")
            )

            def mix_chunk(mq, i):
                blk2, half = i // 2, i % 2
                emit_mix_half(mq, blk2, half, ps_pm)

            for stage in range(4):
                q, mq = stage, stage - 1
                # flipA(mq) first: ready now, so the xbar starts instantly
                # and finishes under the dense step1 burst that follows
                if q == 2:
                    emit_psi_loads(3)  # before the xbar dispatches: transfers
                    # start ahead of the transpose descriptor storm
                if mq >= 0:
                    emit_flipA(mq)
                if mq >= 1:  # flipB deferred a stage: t3g needed in phase B only
                    emit_flipB(mq - 1, 0)
                    emit_flipB(mq - 1, 1)
                if q == 0:
                    emit_psi_loads(0)
                    load_rest_of_consts()
                    emit_psi_loads(1)
                if q == 1:
                    emit_psi_loads(2)
                t1q = t1_pool.tile([128, 256, 32], F16, tag="t1q", name="t1q_t")
                nc.gpsimd.memset(t1q[:, :, 20:32], 0.0)  # zero pad slots
                t1qs[q] = t1q
                if stage == 2:
                    emit_rt_load(0)
                if stage == 3:
                    emit_rt_load(1)
                    emit_rt_load(2)
                    emit_rt_load(3)
                # dense step1 burst first (keeps PE warm and covers the
                # xbar), then all of stage mq's mixes at the stage end
                for b in range(5):
                    emit_s1_b(q, b, ps_s1)
                if mq >= 0:
                    for i in range(4):
                        mix_chunk(mq, i)

            st_s1.close()
            ps_s4a = st_a.enter_context(
                tc.tile_pool(name="ps_s4a", bufs=1, space="PSUM", side="left")
            )
            ps4a = []
            for i in range(4):
                pta = ps_s4a.tile([128, 512], F32, tag=f"s4a_{i}", name=f"ps4a_{i}")
                ps4a.append(pta)

            # ---- stage 4: flipA(3) + kh=0 prologue (blocks 0-2) + mixes(3)
            emit_flipA(3)
            emit_flipB(2, 0)
            emit_flipB(2, 1)
            for blk in range(4):
                for ij in range(4):
                    for f in range(5):
                        s4_mm(ps4a[ij], blk, ij, f, 0, blk == 0 and f == 0, False)
            for i in range(4):
                mix_chunk(3, i)

            st_pm.close()
            ps_s4b = st_b.enter_context(
                tc.tile_pool(name="ps_s4b", bufs=1, space="PSUM", side="right")
            )
            ps4b = []
            for i in range(4):
                ptb = ps_s4b.tile([128, 512], F32, tag=f"s4b_{i}", name=f"ps4b_{i}")
                ps4b.append(ptb)

            # ---- phase B rest: kh=1 catches up on blocks 0-2, then both
            emit_flipB(3, 0)
            emit_flipB(3, 1)
            for blk in range(7):
                if blk < 4:
                    emit_rt_load(blk + 4)
                for ij in range(4):
                    for f in range(5):
                        if blk >= 4:
                            s4_mm(ps4a[ij], blk, ij, f, 0, False, False)
                        s4_mm(ps4b[ij], blk, ij, f, 1, blk == 0 and f == 0, False)
            # final pass: finish one bank at a time so its evac + DMA-out
            # overlaps the remaining banks' matmuls
            for ij in range(4):
                for kh in range(2):
                    ps4t = ps4a[ij] if kh == 0 else ps4b[ij]
                    for f in range(5):
                        s4_mm(ps4t, 7, ij, f, kh, False, f == 4)
                    st = res_pool.tile([128, 512], F32, tag="resst", name="resst_t")
                    evac_copy(st[:], ps4t[:])
                    nc.sync.dma_start(
                        res.ap()[:, ij * 1024 + kh * 512:ij * 1024 + (kh + 1) * 512],
                        st[:],
                    )
            st_a.close()
            st_b.close()
    nc.compile()
    return nc


def _host_inputs(psi_flat, L, M1, M2, R):
    # psi[a,ce,g] -> [ac, q, a_lo, ce, g256]
    psi = np.ascontiguousarray(
        psi_flat.reshape(8, 128, 4, 4, 256).transpose(0, 3, 1, 2, 4)
    ).astype(np.float16)
    # R[f,k,g] -> RT[f,g,k] -> [blk, g_lo, f, k]
    RT = np.ascontiguousarray(
        R.transpose(2, 0, 1).reshape(8, 128, 5, 1024)
    ).astype(np.float16)
    Q = np.einsum("bdic,dfje->bceijf", M1, M2).reshape(20, 20).astype(np.float32)
    # Q4P: rows (g4, slot32) with slot = bce (20 used); cols (ijf, g4)
    Q4P = np.zeros((128, 128), np.float32)
    rows = np.arange(20)
    for g in range(4):
        Q4P[np.ix_(g * 32 + rows, rows * 4 + g)] = Q
    q4_16 = Q4P.astype(np.float16)
    in_maps = []
    for c in range(NCORES):
        LT = np.ascontiguousarray(
            L[:, c * H:(c + 1) * H, :].transpose(0, 2, 1).reshape(5, 8, 128, H)
            .transpose(0, 2, 1, 3)
        ).astype(np.float16)  # [b, a_lo, ac, h]
        in_maps.append({"psi": psi, "lt": LT, "rt": RT, "q4": q4_16})
    return in_maps


def kernel(**inputs):
    psi_flat = np.asarray(inputs["psi_flat"], np.float32)
    L = np.asarray(inputs["L"], np.float32)
    M1 = np.asarray(inputs["M1"], np.float32)
    M2 = np.asarray(inputs["M2"], np.float32)
    R = np.asarray(inputs["R"], np.float32)

    global _nc_cache
    if _nc_cache is None:
        _nc_cache = _build_nc()
    nc = _nc_cache

    in_maps = _host_inputs(psi_flat, L, M1, M2, R)
    out = bass_utils.run_bass_kernel_spmd(nc, in_maps, core_ids=list(range(NCORES)))
    parts = [out.results[c]["res"] for c in range(NCORES)]
    return np.concatenate(parts, axis=0).reshape(-1)


# revision 23
# speedup vs baseline: 1.0045x; 1.0045x over previous
"""DMRG two-site effective Hamiltonian application (ApplyMPO) on 8 trn2 cores.

Math (reference):
  res[h,i,j,k] = sum_{a,b,c,d,e,f,g} L[b,h,a] M1[b,d,i,c] M2[d,f,j,e]
                                     R[f,k,g] psi[a,c,e,g]

Device algorithm (per core, output bond h sharded 8 x 128), all fp16 with
fp32 PSUM accumulation:
  Q[(b,c,e),(i,j,f)] = sum_d M1[b,d,i,c] M2[d,f,j,e]            (host, 400 els)
  step1: T1[h; g, slot32]  = sum_a L[b,h,a] psi[a,(c,e),g]      (PE, K=a)
         slot = b*4 + ce (20 used, 12 zero-padded so 4 g's = 128 cols)
  flipA: T1P[(g4,slot32); w, h] = DMA-xbar transpose, 32 windows per g-block
  mix:   T3H[h; (i,j,f), g]     = T1P[w]^T @ Q4P  (Q4P = I4 (x) Q, padded)
  flipB: T3G[g; (i,j,f), h]     = DMA-xbar transpose (batched, 1 per block)
  step4: res[h; (i,j),k]       += T3G^T @ R^T[f][g,k]  (PE, K=g, 8 PSUM banks
         accumulate across all 8 g-blocks, evacuated once at the end)
Phase A (step1+flip+mix) is a 2-stage software pipeline: mid-section of
quarter q-1 interleaves with step1 of quarter q so PE matmul bursts hide the
DMA-transpose and evacuation latencies. Transposes run on the otherwise-idle
DMA engines (xbar), not the PE.
"""

import numpy as np

import concourse.bacc as bacc
import concourse.mybir as mybir
import concourse.tile as tile
from concourse import bass_utils

F32 = mybir.dt.float32
F16 = mybir.dt.float16

CHI = 1024
W = 5
D = 2
NCORES = 8
H = CHI // NCORES  # 128, h rows per core

_nc_cache = None


def _build_nc():
    nc = bacc.Bacc("TRN2", target_bir_lowering=False)
    # host-prearranged: psi[ac, q, a_lo, ce, g256]; lt[b, a_lo, ac, h]; rt[blk, g_lo, f, k]
    psi = nc.dram_tensor("psi", [8, 4, 128, 4, 256], F16, kind="ExternalInput")
    lt = nc.dram_tensor("lt", [5, 128, 8, H], F16, kind="ExternalInput")
    rt = nc.dram_tensor("rt", [8, 128, 5, 1024], F16, kind="ExternalInput")
    q4 = nc.dram_tensor("q4", [128, 128], F16, kind="ExternalInput")
    res = nc.dram_tensor("res", [H, 4096], F32, kind="ExternalOutput")  # h;(i,j,k)

    with tile.TileContext(nc) as tc:
        with (
            tc.tile_pool(name="const", bufs=1) as const_pool,
            tc.tile_pool(name="psis", bufs=16) as psi_pool,
            tc.tile_pool(name="t1", bufs=2) as t1_pool,
            tc.tile_pool(name="t1p", bufs=3) as t1p_pool,
            tc.tile_pool(name="t3h", bufs=4) as t3h_pool,
            tc.tile_pool(name="t3g", bufs=8) as t3g_pool,
            tc.tile_pool(name="rblk", bufs=4) as rblk_pool,
            tc.tile_pool(name="resst", bufs=2) as res_pool,
        ):
            # ---- static loads (only b=0 weights gate the first matmul) ----
            lt_sb = const_pool.tile([128, 5, 8, H], F16)  # [a_lo; b, ac, h]
            lt_r = lt.ap().rearrange("b p ac h -> p b ac h")
            nc.sync.dma_start(lt_sb[:, 0], lt_r[:, 0])
            q4_sb = const_pool.tile([128, 128], F16)

            def load_rest_of_consts():
                for b in range(1, 5):
                    nc.sync.dma_start(lt_sb[:, b], lt_r[:, b])
                nc.sync.dma_start(q4_sb[:], q4.ap())

            alt_ct = 0

            def evac_copy(out, in_):
                nonlocal alt_ct
                alt_ct += 1
                if alt_ct % 2 == 0:
                    nc.scalar.copy(out, in_)
                else:
                    nc.vector.tensor_copy(out, in_)

            psis = {}     # q -> [8 tiles]
            t1qs = {}     # q -> tile [128, 256, 32]
            t1ps = {}     # (q, blk2) -> tile [128, 32, 128]
            t3hs = {}     # (q, blk2) -> tile [128, 20, 128]
            t3gs = {}     # blk -> tile [128, 20, 128]
            rblks = {}    # blk -> tile [128, 5, 1024]

            def emit_psi_loads(q):
                tiles = []
                for ac in range(8):
                    p = psi_pool.tile([128, 4, 256], F16, tag="psi", name="psi_sl")
                    nc.sync.dma_start(p[:], psi.ap()[ac, q])
                    tiles.append(p)
                psis[q] = tiles

            def emit_rt_load(blk):
                rb = rblk_pool.tile([128, 5, 1024], F16, tag="rblk", name="rblk_t")
                nc.sync.dma_start(rb[:], rt.ap()[blk])
                rblks[blk] = rb

            def xpose(out, in_):
                # NOTE: ACT-dispatched xbar transposes produced garbage on HW
                # (sim-correct); keep every transpose on the SP sequencer.
                nc.sync.dma_start_transpose(out, in_)

            def emit_s1_b(q, b, ps_s1):
                t1q = t1qs[q]
                ps1 = ps_s1.tile([128, 4, 256], F32, tag="s1", name="ps1_t")  # 2 banks
                ps1_flat = ps1[:].rearrange("p c g -> p (c g)")
                for ac in range(8):
                    lhsT = lt_sb[:, b, ac]
                    psi_flat = psis[q][ac][:].rearrange("p c g -> p (c g)")
                    for cep in range(2):  # one 512-wide MM per PSUM bank
                        nc.tensor.matmul(
                            ps1_flat[:, cep * 512:(cep + 1) * 512],
                            lhsT,
                            psi_flat[:, cep * 512:(cep + 1) * 512],
                            start=(ac == 0),
                            stop=(ac == 7),
                        )
                # slot = b*4 + ce; slots 20..31 stay zero (memset at alloc)
                nc.vector.tensor_copy(
                    t1q[:, :, b * 4:b * 4 + 2],
                    ps1[:, 0:2].rearrange("p ce g -> p g ce"),
                )
                nc.scalar.copy(
                    t1q[:, :, b * 4 + 2:b * 4 + 4],
                    ps1[:, 2:4].rearrange("p ce g -> p g ce"),
                )

            def emit_flipA(q):
                # split into 4 xbar pieces per g-block so mixes can start on
                # the first 8 windows instead of waiting for the full block
                t1q = t1qs[q]
                for blk2 in range(2):
                    tp = t1p_pool.tile([128, 32, 128], F16, tag="t1p", name="t1p_t")
                    for pc in range(4):
                        xpose(
                            tp[:, pc * 8:(pc + 1) * 8, :],
                            t1q[:, blk2 * 128 + pc * 32:blk2 * 128 + (pc + 1) * 32, :],
                        )
                    t1ps[(q, blk2)] = tp

            def emit_mix_half(q, blk2, half, ps_pm):
                tp = t1ps[(q, blk2)]
                if half == 0:
                    t3hs[(q, blk2)] = t3h_pool.tile([128, 20, 128], F16, tag="t3h", name="t3h_t")
                t3h = t3hs[(q, blk2)]
                for pair in range(8):  # 2 windows (of 4 g each) per PSUM bank
                    w0 = half * 16 + pair * 2
                    pm = ps_pm.tile([128, 2, 80], F32, tag="pm", name="pm_t")
                    for wi in range(2):
                        nc.tensor.matmul(
                            pm[:, wi],
                            tp[:, w0 + wi, :],
                            q4_sb[:, 0:80],
                            start=True,
                            stop=True,
                        )
                    evac_copy(
                        t3h[:, :, w0 * 4:(w0 + 2) * 4].rearrange(
                            "p i (w g) -> p i w g", w=2
                        ),
                        pm[:].rearrange("p w (i g) -> p i w g", g=4),
                    )

            def emit_flipB(q, blk2):
                blk = q * 2 + blk2
                tg = t3g_pool.tile([128, 20, 128], F16, tag="t3g", name="t3g_t")
                for pc in range(2):
                    xpose(
                        tg[:, pc * 10:(pc + 1) * 10, :],
                        t3hs[(q, blk2)][:, pc * 10:(pc + 1) * 10, :],
                    )
                t3gs[blk] = tg

            # ============ phase A + staged phase-B via side-split PSUM ======
            # s1 lives on PSUM-left, pm on PSUM-right. After stage 3 the s1
            # pool closes and the kh=0 step4 accumulators open on the left, so
            # blocks 0-2 of step4 run while stage 4's mixes trickle behind the
            # flipA(3) xbar. After stage 4 the pm pool closes and the kh=1
            # accumulators take the right side.
            from contextlib import ExitStack as _ES

            def s4_mm(ps4t, blk, ij, f, kh, start, stop):
                nc.tensor.matmul(
                    ps4t[:],
                    t3gs[blk][:, ij * 5 + f, :],
                    rblks[blk][:, f, kh * 512:(kh + 1) * 512],
                    start=start,
                    stop=stop,
                )

            st_s1, st_pm, st_a, st_b = _ES(), _ES(), _ES(), _ES()
            ps_pm = st_pm.enter_context(
                tc.tile_pool(name="ps_pm", bufs=4, space="PSUM", side="right")
            )
            ps_s1 = st_s1.enter_context(
                tc.tile_pool(name="ps_s1", bufs=2, space="PSUM", side="left")
            )

            def mix_chunk(mq, i):
                blk2, half = i // 2, i % 2
                emit_mix_half(mq, blk2, half, ps_pm)

            for stage in range(4):
                q, mq = stage, stage - 1
                # flipA(mq) first: ready now, so the xbar starts instantly
                # and finishes under the dense step1 burst that follows
                if q == 2:
                    emit_psi_loads(3)  # before the xbar dispatches: transfers
                    # start ahead of the transpose descriptor storm
                if mq >= 0:
                    emit_flipA(mq)
                if mq >= 1:  # flipB deferred a stage: t3g needed in phase B only
                    emit_flipB(mq - 1, 0)
                    emit_flipB(mq - 1, 1)
                if q == 0:
                    emit_psi_loads(0)
                    load_rest_of_consts()
                    emit_psi_loads(1)
                if q == 1:
                    emit_psi_loads(2)
                t1q = t1_pool.tile([128, 256, 32], F16, tag="t1q", name="t1q_t")
                nc.gpsimd.memset(t1q[:, :, 20:32], 0.0)  # zero pad slots
                t1qs[q] = t1q
                if stage == 2:
                    emit_rt_load(0)
                if stage == 3:
                    emit_rt_load(1)
                    emit_rt_load(2)
                # dense step1 burst first (keeps PE warm and covers the
                # xbar), then all of stage mq's mixes at the stage end
                for b in range(5):
                    emit_s1_b(q, b, ps_s1)
                if mq >= 0:
                    for i in range(4):
                        mix_chunk(mq, i)

            st_s1.close()
            ps_s4a = st_a.enter_context(
                tc.tile_pool(name="ps_s4a", bufs=1, space="PSUM", side="left")
            )
            ps4a = []
            for i in range(4):
                pta = ps_s4a.tile([128, 512], F32, tag=f"s4a_{i}", name=f"ps4a_{i}")
                ps4a.append(pta)

            # ---- stage 4: flipA(3) + kh=0 prologue (blocks 0-2) + mixes(3)
            emit_flipA(3)
            emit_flipB(2, 0)
            emit_flipB(2, 1)
            emit_rt_load(3)
            for blk in range(3):
                for ij in range(4):
                    for f in range(5):
                        s4_mm(ps4a[ij], blk, ij, f, 0, blk == 0 and f == 0, False)
            for i in range(4):
                mix_chunk(3, i)

            st_pm.close()
            ps_s4b = st_b.enter_context(
                tc.tile_pool(name="ps_s4b", bufs=1, space="PSUM", side="right")
            )
            ps4b = []
            for i in range(4):
                ptb = ps_s4b.tile([128, 512], F32, tag=f"s4b_{i}", name=f"ps4b_{i}")
                ps4b.append(ptb)

            # ---- phase B rest: kh=1 catches up on blocks 0-2, then both
            emit_flipB(3, 0)
            emit_flipB(3, 1)
            for blk in range(7):
                if blk < 4:
                    emit_rt_load(blk + 4)
                for ij in range(4):
                    for f in range(5):
                        if blk >= 3:
                            s4_mm(ps4a[ij], blk, ij, f, 0, False, False)
                        s4_mm(ps4b[ij], blk, ij, f, 1, blk == 0 and f == 0, False)
            # final pass: finish one bank at a time so its evac + DMA-out
            # overlaps the remaining banks' matmuls
            for ij in range(4):
                for kh in range(2):
                    ps4t = ps4a[ij] if kh == 0 else ps4b[ij]
                    for f in range(5):
                        s4_mm(ps4t, 7, ij, f, kh, False, f == 4)
                    st = res_pool.tile([128, 512], F32, tag="resst", name="resst_t")
                    evac_copy(st[:], ps4t[:])
                    nc.sync.dma_start(
                        res.ap()[:, ij * 1024 + kh * 512:ij * 1024 + (kh + 1) * 512],
                        st[:],
                    )
            st_a.close()
            st_b.close()
    nc.compile()
    return nc


def _host_inputs(psi_flat, L, M1, M2, R):
    # psi[a,ce,g] -> [ac, q, a_lo, ce, g256]
    psi = np.ascontiguousarray(
        psi_flat.reshape(8, 128, 4, 4, 256).transpose(0, 3, 1, 2, 4)
    ).astype(np.float16)
    # R[f,k,g] -> RT[f,g,k] -> [blk, g_lo, f, k]
    RT = np.ascontiguousarray(
        R.transpose(2, 0, 1).reshape(8, 128, 5, 1024)
    ).astype(np.float16)
    Q = np.einsum("bdic,dfje->bceijf", M1, M2).reshape(20, 20).astype(np.float32)
    # Q4P: rows (g4, slot32) with slot = bce (20 used); cols (ijf, g4)
    Q4P = np.zeros((128, 128), np.float32)
    rows = np.arange(20)
    for g in range(4):
        Q4P[np.ix_(g * 32 + rows, rows * 4 + g)] = Q
    q4_16 = Q4P.astype(np.float16)
    in_maps = []
    for c in range(NCORES):
        LT = np.ascontiguousarray(
            L[:, c * H:(c + 1) * H, :].transpose(0, 2, 1).reshape(5, 8, 128, H)
            .transpose(0, 2, 1, 3)
        ).astype(np.float16)  # [b, a_lo, ac, h]
        in_maps.append({"psi": psi, "lt": LT, "rt": RT, "q4": q4_16})
    return in_maps


def kernel(**inputs):
    psi_flat = np.asarray(inputs["psi_flat"], np.float32)
    L = np.asarray(inputs["L"], np.float32)
    M1 = np.asarray(inputs["M1"], np.float32)
    M2 = np.asarray(inputs["M2"], np.float32)
    R = np.asarray(inputs["R"], np.float32)

    global _nc_cache
    if _nc_cache is None:
        _nc_cache = _build_nc()
    nc = _nc_cache

    in_maps = _host_inputs(psi_flat, L, M1, M2, R)
    out = bass_utils.run_bass_kernel_spmd(nc, in_maps, core_ids=list(range(NCORES)))
    parts = [out.results[c]["res"] for c in range(NCORES)]
    return np.concatenate(parts, axis=0).reshape(-1)


# revision 24
# speedup vs baseline: 1.0505x; 1.0458x over previous
"""DMRG two-site effective Hamiltonian application (ApplyMPO) on 8 trn2 cores.

Math (reference):
  res[h,i,j,k] = sum_{a,b,c,d,e,f,g} L[b,h,a] M1[b,d,i,c] M2[d,f,j,e]
                                     R[f,k,g] psi[a,c,e,g]

Device algorithm (per core, output bond h sharded 8 x 128), all fp16 with
fp32 PSUM accumulation:
  Q[(b,c,e),(i,j,f)] = sum_d M1[b,d,i,c] M2[d,f,j,e]            (host, 400 els)
  step1: T1[h; g, slot32]  = sum_a L[b,h,a] psi[a,(c,e),g]      (PE, K=a)
         slot = b*4 + ce (20 used, 12 zero-padded so 4 g's = 128 cols)
  flipA: T1P[(g4,slot32); w, h] = DMA-xbar transpose, 32 windows per g-block
  mix:   T3H[h; (i,j,f), g]     = T1P[w]^T @ Q4P  (Q4P = I4 (x) Q, padded)
  flipB: T3G[g; (i,j,f), h]     = DMA-xbar transpose (batched, 1 per block)
  step4: res[h; (i,j),k]       += T3G^T @ R^T[f][g,k]  (PE, K=g, 8 PSUM banks
         accumulate across all 8 g-blocks, evacuated once at the end)
Phase A (step1+flip+mix) is a 2-stage software pipeline: mid-section of
quarter q-1 interleaves with step1 of quarter q so PE matmul bursts hide the
DMA-transpose and evacuation latencies. Transposes run on the otherwise-idle
DMA engines (xbar), not the PE.
"""

import numpy as np

import concourse.bacc as bacc
import concourse.mybir as mybir
import concourse.tile as tile
from concourse import bass_utils

F32 = mybir.dt.float32
F16 = mybir.dt.float16

CHI = 1024
W = 5
D = 2
NCORES = 8
H = CHI // NCORES  # 128, h rows per core

_nc_cache = None


def _build_nc():
    nc = bacc.Bacc("TRN2", target_bir_lowering=False)
    # host-prearranged: psi[ac, q, a_lo, ce, g256]; lt[b, a_lo, ac, h]; rt[blk, g_lo, f, k]
    psi = nc.dram_tensor("psi", [8, 4, 128, 4, 256], F16, kind="ExternalInput")
    lt = nc.dram_tensor("lt", [5, 128, 8, H], F16, kind="ExternalInput")
    rt = nc.dram_tensor("rt", [8, 128, 5, 1024], F16, kind="ExternalInput")
    q4 = nc.dram_tensor("q4", [128, 128], F16, kind="ExternalInput")
    res = nc.dram_tensor("res", [H, 4096], F32, kind="ExternalOutput")  # h;(i,j,k)

    with tile.TileContext(nc) as tc:
        with (
            tc.tile_pool(name="const", bufs=1) as const_pool,
            tc.tile_pool(name="psis", bufs=16) as psi_pool,
            tc.tile_pool(name="t1", bufs=2) as t1_pool,
            tc.tile_pool(name="t1p", bufs=3) as t1p_pool,
            tc.tile_pool(name="t3h", bufs=4) as t3h_pool,
            tc.tile_pool(name="t3g", bufs=8) as t3g_pool,
            tc.tile_pool(name="rblk", bufs=4) as rblk_pool,
            tc.tile_pool(name="resst", bufs=3) as res_pool,
        ):
            # ---- static loads (only b=0 weights gate the first matmul) ----
            lt_sb = const_pool.tile([128, 5, 8, H], F16)  # [a_lo; b, ac, h]
            lt_r = lt.ap().rearrange("b p ac h -> p b ac h")
            nc.sync.dma_start(lt_sb[:, 0], lt_r[:, 0])
            q4_sb = const_pool.tile([128, 128], F16)

            def load_rest_of_consts():
                for b in range(1, 5):
                    nc.sync.dma_start(lt_sb[:, b], lt_r[:, b])
                nc.sync.dma_start(q4_sb[:], q4.ap())

            alt_ct = 0

            def evac_copy(out, in_):
                nonlocal alt_ct
                alt_ct += 1
                if alt_ct % 2 == 0:
                    nc.scalar.copy(out, in_)
                else:
                    nc.vector.tensor_copy(out, in_)

            psis = {}     # q -> [8 tiles]
            t1qs = {}     # q -> tile [128, 256, 32]
            t1ps = {}     # (q, blk2) -> tile [128, 32, 128]
            t3hs = {}     # (q, blk2) -> tile [128, 20, 128]
            t3gs = {}     # blk -> tile [128, 20, 128]
            rblks = {}    # blk -> tile [128, 5, 1024]

            def emit_psi_loads(q):
                tiles = []
                for ac in range(8):
                    p = psi_pool.tile([128, 4, 256], F16, tag="psi", name="psi_sl")
                    nc.sync.dma_start(p[:], psi.ap()[ac, q])
                    tiles.append(p)
                psis[q] = tiles

            def emit_rt_load(blk):
                rb = rblk_pool.tile([128, 5, 1024], F16, tag="rblk", name="rblk_t")
                nc.sync.dma_start(rb[:], rt.ap()[blk])
                rblks[blk] = rb

            def xpose(out, in_):
                # NOTE: ACT-dispatched xbar transposes produced garbage on HW
                # (sim-correct); keep every transpose on the SP sequencer.
                nc.sync.dma_start_transpose(out, in_)

            def emit_s1_b(q, b, ps_s1):
                t1q = t1qs[q]
                ps1 = ps_s1.tile([128, 4, 256], F32, tag="s1", name="ps1_t")  # 2 banks
                ps1_flat = ps1[:].rearrange("p c g -> p (c g)")
                for ac in range(8):
                    lhsT = lt_sb[:, b, ac]
                    psi_flat = psis[q][ac][:].rearrange("p c g -> p (c g)")
                    for cep in range(2):  # one 512-wide MM per PSUM bank
                        nc.tensor.matmul(
                            ps1_flat[:, cep * 512:(cep + 1) * 512],
                            lhsT,
                            psi_flat[:, cep * 512:(cep + 1) * 512],
                            start=(ac == 0),
                            stop=(ac == 7),
                        )
                # slot = b*4 + ce; slots 20..31 stay zero (memset at alloc)
                nc.vector.tensor_copy(
                    t1q[:, :, b * 4:b * 4 + 2],
                    ps1[:, 0:2].rearrange("p ce g -> p g ce"),
                )
                nc.scalar.copy(
                    t1q[:, :, b * 4 + 2:b * 4 + 4],
                    ps1[:, 2:4].rearrange("p ce g -> p g ce"),
                )

            def emit_flipA(q):
                # split into 4 xbar pieces per g-block so mixes can start on
                # the first 8 windows instead of waiting for the full block
                t1q = t1qs[q]
                for blk2 in range(2):
                    tp = t1p_pool.tile([128, 32, 128], F16, tag="t1p", name="t1p_t")
                    for pc in range(4):
                        xpose(
                            tp[:, pc * 8:(pc + 1) * 8, :],
                            t1q[:, blk2 * 128 + pc * 32:blk2 * 128 + (pc + 1) * 32, :],
                        )
                    t1ps[(q, blk2)] = tp

            def emit_mix_half(q, blk2, half, ps_pm):
                tp = t1ps[(q, blk2)]
                if half == 0:
                    t3hs[(q, blk2)] = t3h_pool.tile([128, 20, 128], F16, tag="t3h", name="t3h_t")
                t3h = t3hs[(q, blk2)]
                for pair in range(8):  # 2 windows (of 4 g each) per PSUM bank
                    w0 = half * 16 + pair * 2
                    pm = ps_pm.tile([128, 2, 80], F32, tag="pm", name="pm_t")
                    for wi in range(2):
                        nc.tensor.matmul(
                            pm[:, wi],
                            tp[:, w0 + wi, :],
                            q4_sb[:, 0:80],
                            start=True,
                            stop=True,
                        )
                    evac_copy(
                        t3h[:, :, w0 * 4:(w0 + 2) * 4].rearrange(
                            "p i (w g) -> p i w g", w=2
                        ),
                        pm[:].rearrange("p w (i g) -> p i w g", g=4),
                    )

            def emit_flipB(q, blk2):
                blk = q * 2 + blk2
                tg = t3g_pool.tile([128, 20, 128], F16, tag="t3g", name="t3g_t")
                for pc in range(2):
                    xpose(
                        tg[:, pc * 10:(pc + 1) * 10, :],
                        t3hs[(q, blk2)][:, pc * 10:(pc + 1) * 10, :],
                    )
                t3gs[blk] = tg

            # ============ phase A + staged phase-B via side-split PSUM ======
            # s1 lives on PSUM-left, pm on PSUM-right. After stage 3 the s1
            # pool closes and the kh=0 step4 accumulators open on the left, so
            # blocks 0-2 of step4 run while stage 4's mixes trickle behind the
            # flipA(3) xbar. After stage 4 the pm pool closes and the kh=1
            # accumulators take the right side.
            from contextlib import ExitStack as _ES

            def s4_mm(ps4t, blk, ij, f, kh, start, stop):
                nc.tensor.matmul(
                    ps4t[:],
                    t3gs[blk][:, ij * 5 + f, :],
                    rblks[blk][:, f, kh * 512:(kh + 1) * 512],
                    start=start,
                    stop=stop,
                )

            st_s1, st_pm, st_a, st_b = _ES(), _ES(), _ES(), _ES()
            ps_pm = st_pm.enter_context(
                tc.tile_pool(name="ps_pm", bufs=4, space="PSUM", side="right")
            )
            ps_s1 = st_s1.enter_context(
                tc.tile_pool(name="ps_s1", bufs=2, space="PSUM", side="left")
            )

            def mix_chunk(mq, i):
                blk2, half = i // 2, i % 2
                emit_mix_half(mq, blk2, half, ps_pm)

            for stage in range(4):
                q, mq = stage, stage - 1
                # flipA(mq) first: ready now, so the xbar starts instantly
                # and finishes under the dense step1 burst that follows
                if q == 2:
                    emit_psi_loads(3)  # before the xbar dispatches: transfers
                    # start ahead of the transpose descriptor storm
                if mq >= 0:
                    emit_flipA(mq)
                if mq >= 1:  # flipB deferred a stage: t3g needed in phase B only
                    emit_flipB(mq - 1, 0)
                    emit_flipB(mq - 1, 1)
                if q == 0:
                    emit_psi_loads(0)
                    load_rest_of_consts()
                    emit_psi_loads(1)
                if q == 1:
                    emit_psi_loads(2)
                t1q = t1_pool.tile([128, 256, 32], F16, tag="t1q", name="t1q_t")
                nc.gpsimd.memset(t1q[:, :, 20:32], 0.0)  # zero pad slots
                t1qs[q] = t1q
                if stage == 2:
                    emit_rt_load(0)
                if stage == 3:
                    emit_rt_load(1)
                    emit_rt_load(2)
                # dense step1 burst first (keeps PE warm and covers the
                # xbar), then all of stage mq's mixes at the stage end
                for b in range(5):
                    emit_s1_b(q, b, ps_s1)
                if mq >= 0:
                    for i in range(4):
                        mix_chunk(mq, i)

            st_s1.close()
            ps_s4a = st_a.enter_context(
                tc.tile_pool(name="ps_s4a", bufs=1, space="PSUM", side="left")
            )
            ps4a = []
            for i in range(4):
                pta = ps_s4a.tile([128, 512], F32, tag=f"s4a_{i}", name=f"ps4a_{i}")
                ps4a.append(pta)

            # ---- stage 4: flipA(3) + kh=0 prologue (blocks 0-2) + mixes(3)
            emit_flipA(3)
            emit_flipB(2, 0)
            emit_flipB(2, 1)
            emit_rt_load(3)
            for blk in range(3):
                for ij in range(4):
                    for f in range(5):
                        s4_mm(ps4a[ij], blk, ij, f, 0, blk == 0 and f == 0, False)
            for i in range(4):
                mix_chunk(3, i)

            st_pm.close()
            ps_s4b = st_b.enter_context(
                tc.tile_pool(name="ps_s4b", bufs=1, space="PSUM", side="right")
            )
            ps4b = []
            for i in range(4):
                ptb = ps_s4b.tile([128, 512], F32, tag=f"s4b_{i}", name=f"ps4b_{i}")
                ps4b.append(ptb)

            # ---- phase B rest: kh=1 catches up on blocks 0-2, then both
            emit_flipB(3, 0)
            emit_flipB(3, 1)
            for blk in range(7):
                if blk < 4:
                    emit_rt_load(blk + 4)
                for ij in range(4):
                    for f in range(5):
                        if blk >= 3:
                            s4_mm(ps4a[ij], blk, ij, f, 0, False, False)
                        s4_mm(ps4b[ij], blk, ij, f, 1, blk == 0 and f == 0, False)
            # final pass: finish one bank at a time so its evac + DMA-out
            # overlaps the remaining banks' matmuls
            for ij in range(4):
                for kh in range(2):
                    ps4t = ps4a[ij] if kh == 0 else ps4b[ij]
                    for f in range(5):
                        s4_mm(ps4t, 7, ij, f, kh, False, f == 4)
                    st = res_pool.tile([128, 512], F32, tag="resst", name="resst_t")
                    evac_copy(st[:], ps4t[:])
                    nc.sync.dma_start(
                        res.ap()[:, ij * 1024 + kh * 512:ij * 1024 + (kh + 1) * 512],
                        st[:],
                    )
            st_a.close()
            st_b.close()
    nc.compile()
    return nc


def _host_inputs(psi_flat, L, M1, M2, R):
    # psi[a,ce,g] -> [ac, q, a_lo, ce, g256]
    psi = np.ascontiguousarray(
        psi_flat.reshape(8, 128, 4, 4, 256).transpose(0, 3, 1, 2, 4)
    ).astype(np.float16)
    # R[f,k,g] -> RT[f,g,k] -> [blk, g_lo, f, k]
    RT = np.ascontiguousarray(
        R.transpose(2, 0, 1).reshape(8, 128, 5, 1024)
    ).astype(np.float16)
    Q = np.einsum("bdic,dfje->bceijf", M1, M2).reshape(20, 20).astype(np.float32)
    # Q4P: rows (g4, slot32) with slot = bce (20 used); cols (ijf, g4)
    Q4P = np.zeros((128, 128), np.float32)
    rows = np.arange(20)
    for g in range(4):
        Q4P[np.ix_(g * 32 + rows, rows * 4 + g)] = Q
    q4_16 = Q4P.astype(np.float16)
    in_maps = []
    for c in range(NCORES):
        LT = np.ascontiguousarray(
            L[:, c * H:(c + 1) * H, :].transpose(0, 2, 1).reshape(5, 8, 128, H)
            .transpose(0, 2, 1, 3)
        ).astype(np.float16)  # [b, a_lo, ac, h]
        in_maps.append({"psi": psi, "lt": LT, "rt": RT, "q4": q4_16})
    return in_maps


def kernel(**inputs):
    psi_flat = np.asarray(inputs["psi_flat"], np.float32)
    L = np.asarray(inputs["L"], np.float32)
    M1 = np.asarray(inputs["M1"], np.float32)
    M2 = np.asarray(inputs["M2"], np.float32)
    R = np.asarray(inputs["R"], np.float32)

    global _nc_cache
    if _nc_cache is None:
        _nc_cache = _build_nc()
    nc = _nc_cache

    in_maps = _host_inputs(psi_flat, L, M1, M2, R)
    out = bass_utils.run_bass_kernel_spmd(nc, in_maps, core_ids=list(range(NCORES)))
    parts = [out.results[c]["res"] for c in range(NCORES)]
    return np.concatenate(parts, axis=0).reshape(-1)


# revision 25
# speedup vs baseline: 1.0586x; 1.0077x over previous
"""DMRG two-site effective Hamiltonian application (ApplyMPO) on 8 trn2 cores.

Math (reference):
  res[h,i,j,k] = sum_{a,b,c,d,e,f,g} L[b,h,a] M1[b,d,i,c] M2[d,f,j,e]
                                     R[f,k,g] psi[a,c,e,g]

Device algorithm (per core, output bond h sharded 8 x 128), all fp16 with
fp32 PSUM accumulation:
  Q[(b,c,e),(i,j,f)] = sum_d M1[b,d,i,c] M2[d,f,j,e]            (host, 400 els)
  step1: T1[h; g, slot32]  = sum_a L[b,h,a] psi[a,(c,e),g]      (PE, K=a)
         slot = b*4 + ce (20 used, 12 zero-padded so 4 g's = 128 cols)
  flipA: T1P[(g4,slot32); w, h] = DMA-xbar transpose, 32 windows per g-block
  mix:   T3H[h; (i,j,f), g]     = T1P[w]^T @ Q4P  (Q4P = I4 (x) Q, padded)
  flipB: T3G[g; (i,j,f), h]     = DMA-xbar transpose (batched, 1 per block)
  step4: res[h; (i,j),k]       += T3G^T @ R^T[f][g,k]  (PE, K=g, 8 PSUM banks
         accumulate across all 8 g-blocks, evacuated once at the end)
Phase A (step1+flip+mix) is a 2-stage software pipeline: mid-section of
quarter q-1 interleaves with step1 of quarter q so PE matmul bursts hide the
DMA-transpose and evacuation latencies. Transposes run on the otherwise-idle
DMA engines (xbar), not the PE.
"""

import numpy as np

import concourse.bacc as bacc
import concourse.mybir as mybir
import concourse.tile as tile
from concourse import bass_utils

F32 = mybir.dt.float32
F16 = mybir.dt.float16

CHI = 1024
W = 5
D = 2
NCORES = 8
H = CHI // NCORES  # 128, h rows per core

_nc_cache = None


def _build_nc():
    nc = bacc.Bacc("TRN2", target_bir_lowering=False)
    # host-prearranged: psi[ac, q, a_lo, ce, g256]; lt[b, a_lo, ac, h]; rt[blk, g_lo, f, k]
    psi = nc.dram_tensor("psi", [8, 4, 128, 4, 256], F16, kind="ExternalInput")
    lt = nc.dram_tensor("lt", [5, 128, 8, H], F16, kind="ExternalInput")
    rt = nc.dram_tensor("rt", [8, 128, 5, 1024], F16, kind="ExternalInput")
    q4 = nc.dram_tensor("q4", [128, 128], F16, kind="ExternalInput")
    res = nc.dram_tensor("res", [H, 4096], F32, kind="ExternalOutput")  # h;(i,j,k)

    with tile.TileContext(nc) as tc:
        with (
            tc.tile_pool(name="const", bufs=1) as const_pool,
            tc.tile_pool(name="psis", bufs=2) as psi_pool,
            tc.tile_pool(name="t1", bufs=2) as t1_pool,
            tc.tile_pool(name="t1p", bufs=3) as t1p_pool,
            tc.tile_pool(name="t3h", bufs=4) as t3h_pool,
            tc.tile_pool(name="t3g", bufs=8) as t3g_pool,
            tc.tile_pool(name="rblk", bufs=4) as rblk_pool,
            tc.tile_pool(name="resst", bufs=3) as res_pool,
        ):
            # ---- static loads (only b=0 weights gate the first matmul) ----
            lt_sb = const_pool.tile([128, 5, 8, H], F16)  # [a_lo; b, ac, h]
            lt_r = lt.ap().rearrange("b p ac h -> p b ac h")
            nc.sync.dma_start(lt_sb[:, 0], lt_r[:, 0])
            q4_sb = const_pool.tile([128, 128], F16)

            def load_rest_of_consts():
                for b in range(1, 5):
                    nc.sync.dma_start(lt_sb[:, b], lt_r[:, b])
                nc.sync.dma_start(q4_sb[:], q4.ap())

            alt_ct = 0

            def evac_copy(out, in_):
                nonlocal alt_ct
                alt_ct += 1
                if alt_ct % 2 == 0:
                    nc.scalar.copy(out, in_)
                else:
                    nc.vector.tensor_copy(out, in_)

            psis = {}     # q -> [8 tiles]
            t1qs = {}     # q -> tile [128, 256, 32]
            t1ps = {}     # (q, blk2) -> tile [128, 32, 128]
            t3hs = {}     # (q, blk2) -> tile [128, 20, 128]
            t3gs = {}     # blk -> tile [128, 20, 128]
            rblks = {}    # blk -> tile [128, 5, 1024]

            def emit_psi_loads(q):
                p = psi_pool.tile([128, 8, 4, 256], F16, tag="psi", name="psi_sl")
                nc.sync.dma_start(
                    p[:], psi.ap()[:, q].rearrange("ac p c g -> p ac c g")
                )
                psis[q] = p

            def emit_rt_load(blk):
                rb = rblk_pool.tile([128, 5, 1024], F16, tag="rblk", name="rblk_t")
                nc.sync.dma_start(rb[:], rt.ap()[blk])
                rblks[blk] = rb

            def xpose(out, in_):
                # NOTE: ACT-dispatched xbar transposes produced garbage on HW
                # (sim-correct); keep every transpose on the SP sequencer.
                nc.sync.dma_start_transpose(out, in_)

            def emit_s1_b(q, b, ps_s1):
                t1q = t1qs[q]
                ps1 = ps_s1.tile([128, 4, 256], F32, tag="s1", name="ps1_t")  # 2 banks
                ps1_flat = ps1[:].rearrange("p c g -> p (c g)")
                for ac in range(8):
                    lhsT = lt_sb[:, b, ac]
                    psi_flat = psis[q][:, ac].rearrange("p c g -> p (c g)")
                    for cep in range(2):  # one 512-wide MM per PSUM bank
                        nc.tensor.matmul(
                            ps1_flat[:, cep * 512:(cep + 1) * 512],
                            lhsT,
                            psi_flat[:, cep * 512:(cep + 1) * 512],
                            start=(ac == 0),
                            stop=(ac == 7),
                        )
                # slot = b*4 + ce; slots 20..31 stay zero (memset at alloc)
                nc.vector.tensor_copy(
                    t1q[:, :, b * 4:b * 4 + 2],
                    ps1[:, 0:2].rearrange("p ce g -> p g ce"),
                )
                nc.scalar.copy(
                    t1q[:, :, b * 4 + 2:b * 4 + 4],
                    ps1[:, 2:4].rearrange("p ce g -> p g ce"),
                )

            def emit_flipA(q):
                # split into 4 xbar pieces per g-block so mixes can start on
                # the first 8 windows instead of waiting for the full block
                t1q = t1qs[q]
                for blk2 in range(2):
                    tp = t1p_pool.tile([128, 32, 128], F16, tag="t1p", name="t1p_t")
                    for pc in range(4):
                        xpose(
                            tp[:, pc * 8:(pc + 1) * 8, :],
                            t1q[:, blk2 * 128 + pc * 32:blk2 * 128 + (pc + 1) * 32, :],
                        )
                    t1ps[(q, blk2)] = tp

            def emit_mix_half(q, blk2, half, ps_pm):
                tp = t1ps[(q, blk2)]
                if half == 0:
                    t3hs[(q, blk2)] = t3h_pool.tile([128, 20, 128], F16, tag="t3h", name="t3h_t")
                t3h = t3hs[(q, blk2)]
                for pair in range(8):  # 2 windows (of 4 g each) per PSUM bank
                    w0 = half * 16 + pair * 2
                    pm = ps_pm.tile([128, 2, 80], F32, tag="pm", name="pm_t")
                    for wi in range(2):
                        nc.tensor.matmul(
                            pm[:, wi],
                            tp[:, w0 + wi, :],
                            q4_sb[:, 0:80],
                            start=True,
                            stop=True,
                        )
                    evac_copy(
                        t3h[:, :, w0 * 4:(w0 + 2) * 4].rearrange(
                            "p i (w g) -> p i w g", w=2
                        ),
                        pm[:].rearrange("p w (i g) -> p i w g", g=4),
                    )

            def emit_flipB(q, blk2):
                blk = q * 2 + blk2
                tg = t3g_pool.tile([128, 20, 128], F16, tag="t3g", name="t3g_t")
                for pc in range(2):
                    xpose(
                        tg[:, pc * 10:(pc + 1) * 10, :],
                        t3hs[(q, blk2)][:, pc * 10:(pc + 1) * 10, :],
                    )
                t3gs[blk] = tg

            # ============ phase A + staged phase-B via side-split PSUM ======
            # s1 lives on PSUM-left, pm on PSUM-right. After stage 3 the s1
            # pool closes and the kh=0 step4 accumulators open on the left, so
            # blocks 0-2 of step4 run while stage 4's mixes trickle behind the
            # flipA(3) xbar. After stage 4 the pm pool closes and the kh=1
            # accumulators take the right side.
            from contextlib import ExitStack as _ES

            def s4_mm(ps4t, blk, ij, f, kh, start, stop):
                nc.tensor.matmul(
                    ps4t[:],
                    t3gs[blk][:, ij * 5 + f, :],
                    rblks[blk][:, f, kh * 512:(kh + 1) * 512],
                    start=start,
                    stop=stop,
                )

            st_s1, st_pm, st_a, st_b = _ES(), _ES(), _ES(), _ES()
            ps_pm = st_pm.enter_context(
                tc.tile_pool(name="ps_pm", bufs=4, space="PSUM", side="right")
            )
            ps_s1 = st_s1.enter_context(
                tc.tile_pool(name="ps_s1", bufs=2, space="PSUM", side="left")
            )

            def mix_chunk(mq, i):
                blk2, half = i // 2, i % 2
                emit_mix_half(mq, blk2, half, ps_pm)

            for stage in range(4):
                q, mq = stage, stage - 1
                # flipA(mq) first: ready now, so the xbar starts instantly
                # and finishes under the dense step1 burst that follows
                if q == 2:
                    emit_psi_loads(3)  # before the xbar dispatches: transfers
                    # start ahead of the transpose descriptor storm
                if mq >= 0:
                    emit_flipA(mq)
                if mq >= 1:  # flipB deferred a stage: t3g needed in phase B only
                    emit_flipB(mq - 1, 0)
                    emit_flipB(mq - 1, 1)
                if q == 0:
                    emit_psi_loads(0)
                    load_rest_of_consts()
                    emit_psi_loads(1)
                if q == 1:
                    emit_psi_loads(2)
                t1q = t1_pool.tile([128, 256, 32], F16, tag="t1q", name="t1q_t")
                nc.gpsimd.memset(t1q[:, :, 20:32], 0.0)  # zero pad slots
                t1qs[q] = t1q
                if stage == 2:
                    emit_rt_load(0)
                if stage == 3:
                    emit_rt_load(1)
                    emit_rt_load(2)
                # dense step1 burst first (keeps PE warm and covers the
                # xbar), then all of stage mq's mixes at the stage end
                for b in range(5):
                    emit_s1_b(q, b, ps_s1)
                if mq >= 0:
                    for i in range(4):
                        mix_chunk(mq, i)

            st_s1.close()
            ps_s4a = st_a.enter_context(
                tc.tile_pool(name="ps_s4a", bufs=1, space="PSUM", side="left")
            )
            ps4a = []
            for i in range(4):
                pta = ps_s4a.tile([128, 512], F32, tag=f"s4a_{i}", name=f"ps4a_{i}")
                ps4a.append(pta)

            # ---- stage 4: flipA(3) + kh=0 prologue (blocks 0-2) + mixes(3)
            emit_flipA(3)
            emit_flipB(2, 0)
            emit_flipB(2, 1)
            emit_rt_load(3)
            for blk in range(3):
                for ij in range(4):
                    for f in range(5):
                        s4_mm(ps4a[ij], blk, ij, f, 0, blk == 0 and f == 0, False)
            for i in range(4):
                mix_chunk(3, i)

            st_pm.close()
            ps_s4b = st_b.enter_context(
                tc.tile_pool(name="ps_s4b", bufs=1, space="PSUM", side="right")
            )
            ps4b = []
            for i in range(4):
                ptb = ps_s4b.tile([128, 512], F32, tag=f"s4b_{i}", name=f"ps4b_{i}")
                ps4b.append(ptb)

            # ---- phase B rest: kh=1 catches up on blocks 0-2, then both
            emit_flipB(3, 0)
            emit_flipB(3, 1)
            for blk in range(7):
                if blk < 4:
                    emit_rt_load(blk + 4)
                for ij in range(4):
                    for f in range(5):
                        if blk >= 3:
                            s4_mm(ps4a[ij], blk, ij, f, 0, False, False)
                        s4_mm(ps4b[ij], blk, ij, f, 1, blk == 0 and f == 0, False)
            # final pass: finish one bank at a time so its evac + DMA-out
            # overlaps the remaining banks' matmuls
            for ij in range(4):
                for kh in range(2):
                    ps4t = ps4a[ij] if kh == 0 else ps4b[ij]
                    for f in range(5):
                        s4_mm(ps4t, 7, ij, f, kh, False, f == 4)
                    st = res_pool.tile([128, 512], F32, tag="resst", name="resst_t")
                    evac_copy(st[:], ps4t[:])
                    nc.sync.dma_start(
                        res.ap()[:, ij * 1024 + kh * 512:ij * 1024 + (kh + 1) * 512],
                        st[:],
                    )
            st_a.close()
            st_b.close()
    nc.compile()
    return nc


def _host_inputs(psi_flat, L, M1, M2, R):
    # psi[a,ce,g] -> [ac, q, a_lo, ce, g256]
    psi = np.ascontiguousarray(
        psi_flat.reshape(8, 128, 4, 4, 256).transpose(0, 3, 1, 2, 4)
    ).astype(np.float16)
    # R[f,k,g] -> RT[f,g,k] -> [blk, g_lo, f, k]
    RT = np.ascontiguousarray(
        R.transpose(2, 0, 1).reshape(8, 128, 5, 1024)
    ).astype(np.float16)
    Q = np.einsum("bdic,dfje->bceijf", M1, M2).reshape(20, 20).astype(np.float32)
    # Q4P: rows (g4, slot32) with slot = bce (20 used); cols (ijf, g4)
    Q4P = np.zeros((128, 128), np.float32)
    rows = np.arange(20)
    for g in range(4):
        Q4P[np.ix_(g * 32 + rows, rows * 4 + g)] = Q
    q4_16 = Q4P.astype(np.float16)
    in_maps = []
    for c in range(NCORES):
        LT = np.ascontiguousarray(
            L[:, c * H:(c + 1) * H, :].transpose(0, 2, 1).reshape(5, 8, 128, H)
            .transpose(0, 2, 1, 3)
        ).astype(np.float16)  # [b, a_lo, ac, h]
        in_maps.append({"psi": psi, "lt": LT, "rt": RT, "q4": q4_16})
    return in_maps


def kernel(**inputs):
    psi_flat = np.asarray(inputs["psi_flat"], np.float32)
    L = np.asarray(inputs["L"], np.float32)
    M1 = np.asarray(inputs["M1"], np.float32)
    M2 = np.asarray(inputs["M2"], np.float32)
    R = np.asarray(inputs["R"], np.float32)

    global _nc_cache
    if _nc_cache is None:
        _nc_cache = _build_nc()
    nc = _nc_cache

    in_maps = _host_inputs(psi_flat, L, M1, M2, R)
    out = bass_utils.run_bass_kernel_spmd(nc, in_maps, core_ids=list(range(NCORES)))
    parts = [out.results[c]["res"] for c in range(NCORES)]
    return np.concatenate(parts, axis=0).reshape(-1)
